# revision 35
# baseline (speedup 1.0000x reference)
"""Trainium2 Bass kernel for the ASBIGCN segment_reduce problem.

Contract: kernel(**inputs) takes the FULL unsharded inputs (as produced by the
problem's setup_inputs) and returns the FULL [64, 70000] float32 output.

Strategy (8 NeuronCores):
  - Batch-parallel over B=64 (8 items per core) for the K=3 transformer/GCN/
    biaffine stack. Activations live in SBUF transposed as [600, 8*256] f32r.
  - Per-item span-sum pooling -> per-core feature block [3000, 8].
  - Device AllGather of the tiny feature matrix, then tensor-parallel FC:
    each core computes [64, 8750] against its column slice of fc_w (bf16).
  - Host concatenates the 8 output slices into [64, 70000].

Stack matmuls run in float32r (1 cyc/row at free>=256, ~1e-4 rel err); the
fc weight/feature path is bf16 (~4e-3, still far under the 2e-2 gate).
Softmax chains are batched 4-wide per item pair with independent PE work
(v projections, natural-layout transposes) issued between scores and prob
transposes so the PE stays fed during the DVE/ACT/Pool softmax pipeline.
"""

import sys

sys.path.insert(0, "/opt/trn_rl_repo")

import math

import ml_dtypes
import numpy as np

import concourse.bass as bass
import concourse.mybir as mybir
import concourse.tile as tile
from concourse import bacc
from concourse.bass_utils import run_bass_kernel_spmd
from concourse.masks import make_identity

F32 = mybir.dt.float32
F32R = mybir.dt.float32r
BF16 = mybir.dt.bfloat16
SDT = mybir.dt.float32r  # stack dtype: float32r = 1cyc/row when free>=256
FCT = BF16               # fc phase dtype (halves the 105MB weight DMA)
AX = mybir.AxisListType.X
EXP = mybir.ActivationFunctionType.Exp
RELU = mybir.ActivationFunctionType.Relu
IDENT = mybir.ActivationFunctionType.Identity
ABS = mybir.ActivationFunctionType.Abs
COPY = mybir.ActivationFunctionType.Copy
MUL = mybir.AluOpType.mult
ADD = mybir.AluOpType.add

NCORES = 8
B, S, D = 64, 256, 600
K = 3
BL = B // NCORES          # items per core
NS = BL * S               # 2048 batched free dim
DT, DP = 5, 120           # d split into 5 tiles of 120
OUT1 = 70000
OSH = OUT1 // NCORES      # 8750 output features per core
FDIM = 5 * D              # 3000
FT, FP = 25, 120          # feature tiles
OCH = 512                 # fc output chunk
NOC = math.ceil(OSH / OCH)

QK_BUFS = 1
NO_CC = False       # debug: replace AllGather with local copy (for TimelineSim)


def _och(i):
    return min(OCH, OSH - i * OCH)


def build_nc():
    nc = bacc.Bacc("TRN2", target_bir_lowering=False, debug=False,
                   num_devices=NCORES)

    # ---------------- DRAM I/O ----------------
    xt0 = nc.dram_tensor("xt0", [D, NS], SDT, kind="ExternalInput")
    gts = nc.dram_tensor("gts", [BL, S, S], SDT, kind="ExternalInput")
    negmask = nc.dram_tensor("negmask", [BL, 1, S], SDT, kind="ExternalInput")
    maskq = nc.dram_tensor("maskq", [2, 128, BL], F32, kind="ExternalInput")
    wspan = nc.dram_tensor("wspan", [BL, 1, S], SDT, kind="ExternalInput")
    houtT = nc.dram_tensor("houtT", [D, BL], FCT, kind="ExternalInput")
    tmp1T = nc.dram_tensor("tmp1T", [D, BL], F32, kind="ExternalInput")
    wq = nc.dram_tensor("wq", [K, D, D], SDT, kind="ExternalInput")
    wv = nc.dram_tensor("wv", [K, D, D], SDT, kind="ExternalInput")
    wo = nc.dram_tensor("wo", [K, D, D], SDT, kind="ExternalInput")
    wffc = nc.dram_tensor("wffc", [D, D], SDT, kind="ExternalInput")
    wlin = nc.dram_tensor("wlin", [D, D], SDT, kind="ExternalInput")
    wbiaff = nc.dram_tensor("wbiaff", [D, D], SDT, kind="ExternalInput")
    ffcb = nc.dram_tensor("ffcb", [D, 1], F32, kind="ExternalInput")
    fcw = nc.dram_tensor("fcw", [FDIM, OSH], FCT, kind="ExternalInput")
    fcb = nc.dram_tensor("fcb", [1, OSH], FCT, kind="ExternalInput")
    out = nc.dram_tensor("out", [B, OSH], F32, kind="ExternalOutput")

    with tile.TileContext(nc) as tc:
        with (
            tc.tile_pool(name="pers", bufs=1) as pers,
            tc.tile_pool(name="fcpers", bufs=1) as fpers,
            tc.tile_pool(name="psum", bufs=2, space="PSUM") as psp,
            tc.tile_pool(name="dram", bufs=1, space="DRAM") as dpool,
        ):
            # ---------------- persistent tiles ----------------
            Xt = [pers.tile([DP, NS], SDT, tag=f"Xt{d}", name=f"Xt{d}") for d in range(DT)]
            Xg = [pers.tile([DP, NS], SDT, tag=f"Xg{d}", name=f"Xg{d}") for d in range(DT)]
            for d in range(DT):
                nc.sync.dma_start(Xt[d][:], xt0[d * DP:(d + 1) * DP, :])

            identF = pers.tile([128, 128], F32, tag="identF")
            make_identity(nc, identF[:])
            identR = pers.tile([128, 128], SDT, tag="identR")
            nc.vector.tensor_copy(identR[:], identF[:])
            onescF = pers.tile([1, 128], F32, tag="onescF")
            nc.vector.memset(onescF[:], 1.0)
            onesc = pers.tile([1, 128], SDT, tag="onesc")
            nc.vector.tensor_copy(onesc[:], onescF[:])

            ffcb_c = [pers.tile([DP, 1], F32, tag=f"ffcb{d}", name=f"ffcb{d}") for d in range(DT)]
            mq_t = [pers.tile([128, BL], F32, tag=f"mqt{qt}", name=f"mqt{qt}")
                    for qt in range(2)]

            # fc-phase persistents (loads deferred past layer-0 weights)
            tmpc = [fpers.tile([DP, BL], F32, tag=f"tmpc{d}", name=f"tmpc{d}")
                    for d in range(DT)]
            tmp1c = [fpers.tile([DP, BL], F32, tag=f"tmp1c{d}", name=f"tmp1c{d}")
                     for d in range(DT)]
            dfc = [fpers.tile([DP, BL], F32, tag=f"dfc{d}", name=f"dfc{d}")
                   for d in range(DT)]
            FAb = fpers.tile([FP, BL, FT], FCT, tag="FAb", name="FAb")
            FA = [FAb[:, :, i] for i in range(FT)]
            fTb = fpers.tile([FP, NCORES * BL, FT], FCT, tag="fTb",
                             name="fTb")
            ones32 = fpers.tile([1, B], F32, tag="ones32")
            nc.vector.memset(ones32[:], 1.0)
            ones = fpers.tile([1, B], FCT, tag="ones")
            nc.vector.tensor_copy(ones[:], ones32[:])
            with (
                tc.tile_pool(name="wattn", bufs=1) as wpool,
                tc.tile_pool(name="wrot", bufs=2) as wrot,
                tc.tile_pool(name="work", bufs=1) as wk_pool,
            ):
                def load_w(pool, src, l=None, tagp=None):
                    """Load a [D, D] pre-transposed weight as DT tiles."""
                    tiles = []
                    for d in range(DT):
                        tg = f"{tagp or src.name}{d}"
                        t = pool.tile([DP, D], SDT, tag=tg, name=tg)
                        ap = src.ap()[l] if l is not None else src.ap()
                        nc.sync.dma_start(t[:], ap[d * DP:(d + 1) * DP, :])
                        tiles.append(t)
                    return tiles

                def ps2k():
                    return psp.tile([128, 2 * S], F32, tag="ps256", bufs=2, name="ps2k")

                def ps1k():
                    return psp.tile([128, S], F32, tag="scps", bufs=4, name="ps1k")

                def softmax(ps, qt=None, j=None):
                    """scores psum -> normalized probs (SBUF). 4 chains in
                    flight: PE->DVE(max)->ACT(neg,exp+Z)->DVE(recip[,mask])
                    ->Pool(scale)."""
                    mx = wk_pool.tile([128, 1], F32, tag="mx", bufs=4, name="mx")
                    nc.vector.reduce_max(mx[:], ps[:], axis=AX)
                    ngm = wk_pool.tile([128, 1], F32, tag="ngm", bufs=4,
                                       name="ngm")
                    nc.scalar.mul(ngm[:], mx[:], -1.0)
                    probs = wk_pool.tile([128, S], F32, tag="probs", bufs=4,
                                         name="probs")
                    Z = wk_pool.tile([128, 1], F32, tag="Z", bufs=4, name="Z")
                    nc.scalar.activation(probs[:], ps[:], EXP, bias=ngm[:],
                                         scale=1.0, accum_out=Z[:])
                    r = wk_pool.tile([128, 1], F32, tag="r", bufs=4, name="r")
                    nc.vector.reciprocal(r[:], Z[:])
                    if qt is not None:
                        rm = wk_pool.tile([128, 1], F32, tag="rm", bufs=4,
                                          name="rm")
                        nc.vector.tensor_mul(rm[:], r[:], mq_t[qt][:, j:j + 1])
                        r = rm
                    nc.gpsimd.tensor_scalar_mul(probs[:], probs[:], r[:])
                    return probs

                drain_rr = [0]

                def drain(dst, src):
                    """PSUM->SBUF drain, rotated 2:1 ACT:DVE so psum rings
                    free even when one queue is backed up."""
                    i = drain_rr[0]
                    drain_rr[0] += 1
                    if i % 3 < 2:
                        nc.scalar.activation(dst, src, COPY)
                    else:
                        nc.vector.tensor_copy(dst, src)

                # ---------------- the 3-layer stack ----------------
                for l in range(K):
                    wq_sb = load_w(wpool, wq, l)   # M = scale * Wq^T Wk
                    wv_sb = load_w(wpool, wv, l)
                    if l == 0:
                        # deferred cold-start loads: first q/k matmuls only
                        # need Xt + wq/wk; everything here is used later
                        for d in range(DT):
                            nc.sync.dma_start(Xg[d][:],
                                              xt0[d * DP:(d + 1) * DP, :])
                        for d in range(DT):
                            nc.sync.dma_start(ffcb_c[d][:],
                                              ffcb[d * DP:(d + 1) * DP, :])
                        for qt in range(2):
                            nc.sync.dma_start(mq_t[qt][:], maskq.ap()[qt])
                        for d in range(DT):
                            nc.sync.dma_start(tmp1c[d][:],
                                              tmp1T[d * DP:(d + 1) * DP, :])
                    wo_sb = load_w(wrot, wo, l, tagp="wrot")
                    wffc_sb = load_w(wrot, wffc, tagp="wrot")

                    # ---- attention (per pair; 4 softmax chains batched) ----
                    for j in range(0, BL, 2):
                        pcols = slice(j * S, (j + 2) * S)
                        qT_t = []
                        for do in range(DT):
                            ps = ps2k()
                            for di in range(DT):
                                nc.tensor.matmul(
                                    ps[:DP, :],
                                    wq_sb[di][:, do * DP:(do + 1) * DP],
                                    Xt[di][:, pcols],
                                    start=(di == 0), stop=(di == DT - 1))
                            t = wk_pool.tile([DP, 2 * S], SDT, tag=f"qT{do}",
                                             name=f"qT{do}", bufs=QK_BUFS)
                            drain(t[:], ps[:DP, :])
                            qT_t.append(t)
                        # scores = (X M) X^T for both items x both q-halves;
                        # exact fp32 in the last layer (scores reach +-300 and
                        # sit pre-softmax: f32r noise there flips near-hardmax
                        # picks)
                        probs_l = {}
                        for jj in range(2):
                            off = jj * S
                            icols = slice((j + jj) * S, (j + jj + 1) * S)
                            for qt in range(2):
                                ps = ps1k()
                                for di in range(DT):
                                    qs = qT_t[di][:, off + qt * 128:
                                                  off + qt * 128 + 128]
                                    rh = Xt[di][:, icols]
                                    if l == K - 1:
                                        qs = qs.bitcast(F32)
                                        rh = rh.bitcast(F32)
                                    nc.tensor.matmul(
                                        ps[:], qs, rh,
                                        start=(di == 0), stop=(di == DT - 1))
                                probs_l[(jj, qt)] = softmax(ps)
                        # v for both items (PE filler during softmax chains)
                        v_pair = []
                        for jj in range(2):
                            v_sb = []
                            for st in range(2):
                                t = wk_pool.tile([128, D], SDT,
                                                 tag=f"v{2 * jj + st}",
                                                 name=f"v{2 * jj + st}")
                                scol = slice((j + jj) * S + st * 128,
                                             (j + jj) * S + st * 128 + 128)
                                for nt in range(2):
                                    ps = ps2k()
                                    for di in range(DT):
                                        nc.tensor.matmul(
                                            ps[:, :300], Xt[di][:, scol],
                                            wv_sb[di][:, nt * 300:(nt + 1) * 300],
                                            start=(di == 0), stop=(di == DT - 1))
                                    drain(t[:, nt * 300:(nt + 1) * 300],
                                          ps[:, :300])
                                v_sb.append(t)
                            v_pair.append(v_sb)
                        # prob transposes (paired drains) + attn + out proj
                        for jj in range(2):
                            cols = slice((j + jj) * S, (j + jj + 1) * S)
                            aTb = wk_pool.tile([128, 2 * S], SDT,
                                               tag=f"aTb{jj}", name=f"aTb{jj}")
                            aTv = aTb[:].rearrange("p (k q) -> p k q", k=2)
                            for qt in range(2):
                                probs = probs_l[(jj, qt)]
                                pt_ps = ps1k()
                                for kt in range(2):
                                    nc.tensor.transpose(
                                        pt_ps[:, kt * 128:(kt + 1) * 128],
                                        probs[:, kt * 128:(kt + 1) * 128],
                                        identF[:])
                                drain(aTv[:, :, qt * 128:(qt + 1) * 128],
                                      pt_ps[:].rearrange("p (k q) -> p k q",
                                                         k=2))
                            attnT = []
                            for d in range(DT):
                                ps = ps1k()
                                for kt in range(2):
                                    nc.tensor.matmul(
                                        ps[:DP, :],
                                        v_pair[jj][kt][:, d * DP:(d + 1) * DP],
                                        aTb[:, kt * S:(kt + 1) * S],
                                        start=(kt == 0), stop=(kt == 1))
                                t = wk_pool.tile([DP, S], SDT, tag=f"attnT{d}",
                                                 name=f"attnT{d}")
                                drain(t[:], ps[:DP, :])
                                attnT.append(t)
                            for do in range(DT):
                                ps = ps1k()
                                for di in range(DT):
                                    nc.tensor.matmul(
                                        ps[:DP, :],
                                        wo_sb[di][:, do * DP:(do + 1) * DP],
                                        attnT[di][:],
                                        start=(di == 0), stop=(di == DT - 1))
                                nc.vector.tensor_add(Xt[do][:, cols],
                                                     Xt[do][:, cols],
                                                     ps[:DP, :])

                    # ---- ffc (per pair; staged to dodge in-place hazard) ----
                    for j in range(0, BL, 2):
                        ccol = slice(j * S, (j + 2) * S)
                        stages = []
                        for do in range(DT):
                            ps = ps2k()
                            for di in range(DT):
                                nc.tensor.matmul(
                                    ps[:DP, :],
                                    wffc_sb[di][:, do * DP:(do + 1) * DP],
                                    Xt[di][:, ccol],
                                    start=(di == 0), stop=(di == DT - 1))
                            st = wk_pool.tile([128, 2 * S], SDT,
                                              tag=f"stg{do}",
                                              name=f"stg{do}")
                            if do % 3 < 2:
                                nc.scalar.activation(st[:DP, :], ps[:DP, :],
                                                     IDENT, bias=ffcb_c[do][:])
                            else:
                                nc.vector.tensor_scalar_add(st[:DP, :],
                                                            ps[:DP, :],
                                                            ffcb_c[do][:])
                            stages.append(st)
                        for do in range(DT):
                            nc.gpsimd.tensor_add(Xt[do][:, ccol],
                                                 Xt[do][:, ccol],
                                                 stages[do][:DP, :])

                    # ---- graph conv (per item): Xg += relu((G/den) @ te) ----
                    wlin_sb = load_w(wrot, wlin, tagp="wrot")
                    for j in range(BL):
                        cols = slice(j * S, (j + 1) * S)
                        te_sb = []
                        for st in range(2):
                            t = wk_pool.tile([128, D], SDT, tag=f"v{st}",
                                             name=f"te{st}")
                            scol = slice(j * S + st * 128,
                                         j * S + st * 128 + 128)
                            for nt in range(2):
                                ps = ps2k()
                                for di in range(DT):
                                    nc.tensor.matmul(
                                        ps[:, :300], Xg[di][:, scol],
                                        wlin_sb[di][:, nt * 300:(nt + 1) * 300],
                                        start=(di == 0), stop=(di == DT - 1))
                                drain(t[:, nt * 300:(nt + 1) * 300],
                                      ps[:, :300])
                            te_sb.append(t)
                        g_sb = []
                        for kt in range(2):
                            t = wk_pool.tile([128, S], SDT, tag=f"sh{kt}",
                                             name=f"g{kt}")
                            nc.sync.dma_start(
                                t[:], gts.ap()[j, kt * 128:(kt + 1) * 128, :])
                            g_sb.append(t)
                        for d in range(DT):
                            ps = ps1k()
                            for kt in range(2):
                                nc.tensor.matmul(
                                    ps[:DP, :],
                                    te_sb[kt][:, d * DP:(d + 1) * DP],
                                    g_sb[kt][:], start=(kt == 0),
                                    stop=(kt == 1))
                            rl = wk_pool.tile([DP, S], F32, tag="rl",
                                              name="rl", bufs=2)
                            nc.scalar.activation(rl[:], ps[:DP, :], RELU)
                            nc.vector.tensor_add(Xg[d][:, cols],
                                                 Xg[d][:, cols], rl[:])

                    # ---- mutual biaffine (per item; chains batched) ----
                    wb_sb = load_w(wrot, wbiaff, tagp="wrot")
                    pqkT = {}
                    for j in range(BL):
                        cols = slice(j * S, (j + 1) * S)
                        if j % 2 == 0:
                            pcols = slice(j * S, (j + 2) * S)
                            pqkT = {"q": [], "k": []}
                            for (xsrc, nm) in ((Xt, "q"), (Xg, "k")):
                                for do in range(DT):
                                    ps = ps2k()
                                    for di in range(DT):
                                        nc.tensor.matmul(
                                            ps[:DP, :],
                                            wb_sb[di][:, do * DP:(do + 1) * DP],
                                            xsrc[di][:, pcols],
                                            start=(di == 0),
                                            stop=(di == DT - 1))
                                    t = wk_pool.tile([DP, 2 * S], SDT,
                                                     tag=f"{nm}T{do}",
                                                     name=f"{nm}T{do}",
                                                     bufs=QK_BUFS)
                                    drain(t[:], ps[:DP, :])
                                    pqkT[nm].append(t)
                        off = (j % 2) * S
                        negrow = wk_pool.tile([1, S], SDT, tag="negrow",
                                              name="negrow", bufs=2)
                        nc.sync.dma_start(negrow[:], negmask.ap()[j])
                        # l1/l2 scores (+rank-1 neg mask preload), 4 chains;
                        # exact fp32 in the last layer (scores reach +-16k;
                        # f32r noise is O(1) absolute there and flips the
                        # effectively-hardmax attention picks)
                        probs_l = {}
                        f32sc = l == K - 1
                        for (pnm, xrhs, nm) in (("q", Xg, "l1"),
                                                ("k", Xt, "l2")):
                            for qt in range(2):
                                ps = ps1k()
                                if f32sc:
                                    nc.tensor.matmul(ps[:], onescF[:, :128],
                                                     negrow[:].bitcast(F32),
                                                     start=True, stop=False)
                                else:
                                    nc.tensor.matmul(ps[:], onesc[:, :128],
                                                     negrow[:], start=True,
                                                     stop=False)
                                for di in range(DT):
                                    pv = pqkT[pnm][di][:, off + qt * 128:
                                                       off + qt * 128 + 128]
                                    rh = xrhs[di][:, cols]
                                    if f32sc:
                                        pv = pv.bitcast(F32)
                                        rh = rh.bitcast(F32)
                                    nc.tensor.matmul(
                                        ps[:], pv, rh,
                                        start=False, stop=(di == DT - 1))
                                probs_l[(nm, qt)] = softmax(ps, qt, j)
                        # natural-layout Xt/Xg (PE filler during chains)
                        natXt, natXg = [], []
                        for (X, nat, base) in ((Xt, natXt, 0), (Xg, natXg, 2)):
                            for st in range(2):
                                t = wk_pool.tile([128, D], SDT,
                                                 tag=f"v{base + st}",
                                                 name=f"nat{base + st}")
                                scol = slice(j * S + st * 128,
                                             j * S + st * 128 + 128)
                                for d0 in range(0, DT, 2):
                                    dn = min(2, DT - d0)
                                    pt_ps = psp.tile([128, S], SDT,
                                                     tag="psT", bufs=2,
                                                     name="psTn")
                                    for dd in range(dn):
                                        nc.tensor.transpose(
                                            pt_ps[:, dd * DP:(dd + 1) * DP],
                                            X[d0 + dd][:, scol],
                                            identR[:DP, :DP])
                                    drain(t[:, d0 * DP:(d0 + dn) * DP],
                                          pt_ps[:, :dn * DP])
                                nat.append(t)
                        # prob transposes (paired drains)
                        lTb = {}
                        for nm in ("l1", "l2"):
                            tb = wk_pool.tile([128, 2 * S], SDT,
                                              tag=f"aTb{(nm == 'l2') * 1}",
                                              name=f"lTb{nm}")
                            tv = tb[:].rearrange("p (k q) -> p k q", k=2)
                            for qt in range(2):
                                probs = probs_l[(nm, qt)]
                                pt_ps = ps1k()
                                for kt in range(2):
                                    nc.tensor.transpose(
                                        pt_ps[:, kt * 128:(kt + 1) * 128],
                                        probs[:, kt * 128:(kt + 1) * 128],
                                        identF[:])
                                drain(tv[:, :, qt * 128:(qt + 1) * 128],
                                      pt_ps[:].rearrange("p (k q) -> p k q",
                                                         k=2))
                            lTb[nm] = tb
                        # o1 into Xt, o2 into Xg (q-mask folded into rm)
                        for (nat, lname, X) in ((natXg, "l1", Xt),
                                                (natXt, "l2", Xg)):
                            for d in range(DT):
                                ps = ps1k()
                                for kt in range(2):
                                    nc.tensor.matmul(
                                        ps[:DP, :],
                                        nat[kt][:, d * DP:(d + 1) * DP],
                                        lTb[lname][:, kt * S:(kt + 1) * S],
                                        start=(kt == 0), stop=(kt == 1))
                                nc.vector.tensor_add(X[d][:, cols],
                                                     X[d][:, cols],
                                                     ps[:DP, :])
                        # span sum of out_t for this item (last layer only)
                        if l == K - 1:
                            ws_bc = wk_pool.tile([128, S], SDT, tag="nmbc",
                                                 name="ws_bc")
                            nc.sync.dma_start(
                                ws_bc[:],
                                wspan.ap()[j].partition_broadcast(128))
                            for d in range(DT):
                                msel = wk_pool.tile([DP, S], F32, tag="msel",
                                                    name="msel")
                                nc.gpsimd.tensor_mul(msel[:], Xt[d][:, cols],
                                                     ws_bc[:DP, :])
                                nc.vector.reduce_sum(tmpc[d][:, j:j + 1],
                                                     msel[:], axis=AX)

                # feature blocks: [hout, tmp, tmp1, tmp*tmp1, |tmp-tmp1|]
                for d in range(DT):
                    nc.gpsimd.dma_start(FA[d], houtT[d * DP:(d + 1) * DP, :])
                    nc.gpsimd.tensor_copy(FA[5 + d], tmpc[d][:])
                    nc.gpsimd.tensor_copy(FA[10 + d], tmp1c[d][:])
                    nc.gpsimd.tensor_mul(FA[15 + d], tmpc[d][:],
                                         tmp1c[d][:])
                    nc.gpsimd.tensor_sub(dfc[d][:], tmpc[d][:], tmp1c[d][:])
                    nc.scalar.activation(FA[20 + d], dfc[d][:], ABS)

            # ---------------- FC: out = feat @ fc_w.T + fc_b ----------------
            with tc.tile_pool(name="fc", bufs=2) as fcp:
                fcw_v = fcw.ap().rearrange("(f p) o -> p f o", p=FP)

                def load_wg(oc):
                    w = _och(oc)
                    wg = fcp.tile([FP, FT, OCH], FCT, tag="wg", name="wg")
                    for f0, f1 in ((0, 7), (7, 13), (13, 19), (19, 25)):
                        nc.sync.dma_start(
                            wg[:, f0:f1, :w],
                            fcw_v[:, f0:f1, oc * OCH:oc * OCH + w])
                    fcbc = fcp.tile([1, OCH], FCT, tag="fcbc", name="fcbc")
                    nc.sync.dma_start(fcbc[:, :w],
                                      fcb[:, oc * OCH:oc * OCH + w])
                    return wg, fcbc

                # prefetch the first two weight chunks; they overlap the
                # feature AllGather below
                wg_pre = [load_wg(0), load_wg(1)]

                # ---------------- allgather features ----------------
                feat_l = dpool.tile([FP, BL, FT], FCT)
                nc.sync.dma_start(feat_l[:], FAb[:])
                feat_g = dpool.tile([NCORES, FP, BL, FT], FCT)
                if NO_CC:
                    nc.sync.dma_start(feat_g[0], feat_l[:])
                else:
                    nc.gpsimd.collective_compute(
                        "AllGather", mybir.AluOpType.bypass,
                        replica_groups=[list(range(NCORES))],
                        ins=[feat_l.opt()], outs=[feat_g.opt()])
                # [c, p, j, f] -> [p, c, j, f]: contiguous (j f) runs of 400B
                gview = feat_g[:].rearrange("c p j f -> p c j f")
                fTv = fTb[:].rearrange("p (c j) f -> p c j f", c=NCORES)
                nc.sync.dma_start(fTv, gview)

                for oc in range(NOC):
                    w = _och(oc)
                    wg, fcbc = wg_pre[oc] if oc < 2 else load_wg(oc)
                    ps = psp.tile([128, OCH], F32, tag="ps256", bufs=2, name="psfc")
                    for i in range(FT):
                        nc.tensor.matmul(
                            ps[:B, :w], fTb[:, :, i],
                            wg[:, i, :w], start=(i == 0), stop=False)
                    nc.tensor.matmul(ps[:B, :w], ones[:], fcbc[:, :w],
                                     start=False, stop=True)
                    ot = fcp.tile([B, OCH], F32, tag="ot", name="ot")
                    nc.vector.tensor_copy(ot[:, :w], ps[:B, :w])
                    nc.sync.dma_start(out[:, oc * OCH:oc * OCH + w],
                                      ot[:, :w])

    nc.compile()
    return nc


def prep_inputs(lstm_out, hout, dependency_graph, attn_in, attn_out, ffc_w,
                ffc_b, lin_w, biaff_w, fc_w, fc_b, text_len, spans):
    """Host-side sharding + layout transforms. Returns per-core input maps."""
    f32 = np.float32
    lstm_out = np.asarray(lstm_out, dtype=f32)
    hout = np.asarray(hout, dtype=f32)
    G = np.asarray(dependency_graph, dtype=f32)
    attn_in = np.asarray(attn_in, dtype=f32)
    attn_out = np.asarray(attn_out, dtype=f32)
    fc_w = np.asarray(fc_w, dtype=f32)
    text_len = np.asarray(text_len)
    spans = np.asarray(spans)

    scale = 1.0 / math.sqrt(D)
    # fold q/k into one projection: scores = (X Wq^T)(Wk X^T) = X M X^T
    wq = np.ascontiguousarray(np.stack([
        (attn_in[l, :D, :].T.astype(np.float64) * scale)
        @ attn_in[l, D:2 * D, :].astype(np.float64)
        for l in range(K)]).astype(f32))
    wv = np.ascontiguousarray(np.stack([attn_in[l, 2 * D:, :].T
                                        for l in range(K)]))
    wo = np.ascontiguousarray(np.stack([attn_out[l].T for l in range(K)]))
    wffc = np.ascontiguousarray(np.asarray(ffc_w, dtype=f32).T)
    wlin = np.ascontiguousarray(np.asarray(lin_w, dtype=f32).T)
    wbiaff = np.ascontiguousarray(np.asarray(biaff_w, dtype=f32).T)
    ffcb = np.ascontiguousarray(np.asarray(ffc_b, dtype=f32).reshape(D, 1))
    fcb = np.asarray(fc_b, dtype=f32).reshape(1, OUT1)

    idx = np.arange(S)
    mask = (idx[None, :] < text_len[:, None].astype(np.int64)).astype(f32)
    negm = (-10000.0 * (1.0 - mask)).reshape(B, 1, S)
    maskq_h = mask.reshape(B, 2, 128)
    s0 = spans[:, 0, 0].astype(np.int64)[:, None]
    e0 = spans[:, 0, 1].astype(np.int64)[:, None]
    wsp = ((idx[None, :] >= s0) & (idx[None, :] < e0)).astype(f32)
    tmp1 = np.einsum('bs,bsd->bd', wsp, lstm_out)  # span_sum(lstm_out)[:, 0]
    wsp = wsp.reshape(B, 1, S)

    denom = G.sum(axis=2, keepdims=True) + 1e-7
    GTs = np.ascontiguousarray((G / denom).transpose(0, 2, 1))

    in_maps = []
    for c in range(NCORES):
        bs = slice(c * BL, (c + 1) * BL)
        xt0 = np.ascontiguousarray(
            lstm_out[bs].transpose(2, 0, 1).reshape(D, NS))
        in_maps.append({
            "xt0": xt0,
            "gts": np.ascontiguousarray(GTs[bs]),
            "negmask": np.ascontiguousarray(negm[bs]),
            "maskq": np.ascontiguousarray(maskq_h[bs].transpose(1, 2, 0)),
            "wspan": np.ascontiguousarray(wsp[bs]),
            "houtT": np.ascontiguousarray(hout[bs].T).astype(ml_dtypes.bfloat16),
            "tmp1T": np.ascontiguousarray(tmp1[bs].T),
            "wq": wq, "wv": wv, "wo": wo,
            "wffc": wffc, "wlin": wlin, "wbiaff": wbiaff, "ffcb": ffcb,
            "fcw": np.ascontiguousarray(
                fc_w[c * OSH:(c + 1) * OSH, :].T).astype(ml_dtypes.bfloat16),
            "fcb": np.ascontiguousarray(
                fcb[:, c * OSH:(c + 1) * OSH]).astype(ml_dtypes.bfloat16),
        })
    return in_maps


_NC = None


def get_nc():
    global _NC
    if _NC is None:
        _NC = build_nc()
    return _NC


def kernel(**inputs) -> np.ndarray:
    nc = get_nc()
    in_maps = prep_inputs(**inputs)
    res = run_bass_kernel_spmd(nc, in_maps, list(range(NCORES)))
    return np.concatenate([res.results[c]["out"] for c in range(NCORES)],
                          axis=1)


# revision 36
# speedup vs baseline: 1.0300x; 1.0300x over previous
"""Trainium2 Bass kernel for the ASBIGCN segment_reduce problem.

Contract: kernel(**inputs) takes the FULL unsharded inputs (as produced by the
problem's setup_inputs) and returns the FULL [64, 70000] float32 output.

Strategy (8 NeuronCores):
  - Batch-parallel over B=64 (8 items per core) for the K=3 transformer/GCN/
    biaffine stack. Activations live in SBUF transposed as [600, 8*256] f32r.
  - Per-item span-sum pooling -> per-core feature block [3000, 8].
  - Device AllGather of the tiny feature matrix, then tensor-parallel FC:
    each core computes [64, 8750] against its column slice of fc_w (bf16).
  - Host concatenates the 8 output slices into [64, 70000].

Stack matmuls run in float32r (1 cyc/row at free>=256, ~1e-4 rel err); the
fc weight/feature path is bf16 (~4e-3, still far under the 2e-2 gate).
Softmax chains are batched 4-wide per item pair with independent PE work
(v projections, natural-layout transposes) issued between scores and prob
transposes so the PE stays fed during the DVE/ACT/Pool softmax pipeline.
"""

import sys

sys.path.insert(0, "/opt/trn_rl_repo")

import math

import ml_dtypes
import numpy as np

import concourse.bass as bass
import concourse.mybir as mybir
import concourse.tile as tile
from concourse import bacc
from concourse.bass_utils import run_bass_kernel_spmd
from concourse.masks import make_identity

F32 = mybir.dt.float32
F32R = mybir.dt.float32r
BF16 = mybir.dt.bfloat16
SDT = mybir.dt.float32r  # stack dtype: float32r = 1cyc/row when free>=256
FCT = BF16               # fc phase dtype (halves the 105MB weight DMA)
AX = mybir.AxisListType.X
EXP = mybir.ActivationFunctionType.Exp
RELU = mybir.ActivationFunctionType.Relu
IDENT = mybir.ActivationFunctionType.Identity
ABS = mybir.ActivationFunctionType.Abs
COPY = mybir.ActivationFunctionType.Copy
MUL = mybir.AluOpType.mult
ADD = mybir.AluOpType.add

NCORES = 8
B, S, D = 64, 256, 600
K = 3
BL = B // NCORES          # items per core
NS = BL * S               # 2048 batched free dim
DT, DP = 5, 120           # d split into 5 tiles of 120
OUT1 = 70000
OSH = OUT1 // NCORES      # 8750 output features per core
FDIM = 5 * D              # 3000
FT, FP = 25, 120          # feature tiles
OCH = 512                 # fc output chunk
NOC = math.ceil(OSH / OCH)

QK_BUFS = 1
NO_CC = False       # debug: replace AllGather with local copy (for TimelineSim)


def _och(i):
    return min(OCH, OSH - i * OCH)


def build_nc():
    nc = bacc.Bacc("TRN2", target_bir_lowering=False, debug=False,
                   num_devices=NCORES)

    # ---------------- DRAM I/O ----------------
    xt0 = nc.dram_tensor("xt0", [D, NS], SDT, kind="ExternalInput")
    gts = nc.dram_tensor("gts", [BL, S, S], SDT, kind="ExternalInput")
    negmask = nc.dram_tensor("negmask", [BL, 1, S], SDT, kind="ExternalInput")
    maskq = nc.dram_tensor("maskq", [2, 128, BL], F32, kind="ExternalInput")
    wspan = nc.dram_tensor("wspan", [BL, 1, S], SDT, kind="ExternalInput")
    houtT = nc.dram_tensor("houtT", [D, BL], FCT, kind="ExternalInput")
    tmp1T = nc.dram_tensor("tmp1T", [D, BL], F32, kind="ExternalInput")
    wq = nc.dram_tensor("wq", [K, D, D], SDT, kind="ExternalInput")
    wv = nc.dram_tensor("wv", [K, D, D], SDT, kind="ExternalInput")
    wo = nc.dram_tensor("wo", [K, D, D], SDT, kind="ExternalInput")
    wffc = nc.dram_tensor("wffc", [D, D], SDT, kind="ExternalInput")
    wlin = nc.dram_tensor("wlin", [D, D], SDT, kind="ExternalInput")
    wbiaff = nc.dram_tensor("wbiaff", [D, D], SDT, kind="ExternalInput")
    ffcb = nc.dram_tensor("ffcb", [D, 1], F32, kind="ExternalInput")
    fcw = nc.dram_tensor("fcw", [FDIM, OSH], FCT, kind="ExternalInput")
    fcb = nc.dram_tensor("fcb", [1, OSH], FCT, kind="ExternalInput")
    out = nc.dram_tensor("out", [B, OSH], F32, kind="ExternalOutput")

    with tile.TileContext(nc) as tc:
        with (
            tc.tile_pool(name="pers", bufs=1) as pers,
            tc.tile_pool(name="fcpers", bufs=1) as fpers,
            tc.tile_pool(name="psum", bufs=2, space="PSUM") as psp,
            tc.tile_pool(name="dram", bufs=1, space="DRAM") as dpool,
        ):
            # ---------------- persistent tiles ----------------
            Xt = [pers.tile([DP, NS], SDT, tag=f"Xt{d}", name=f"Xt{d}") for d in range(DT)]
            Xg = [pers.tile([DP, NS], SDT, tag=f"Xg{d}", name=f"Xg{d}") for d in range(DT)]
            for d in range(DT):
                nc.sync.dma_start(Xt[d][:], xt0[d * DP:(d + 1) * DP, :])

            identF = pers.tile([128, 128], F32, tag="identF")
            make_identity(nc, identF[:])
            identR = pers.tile([128, 128], SDT, tag="identR")
            nc.vector.tensor_copy(identR[:], identF[:])
            onescF = pers.tile([1, 128], F32, tag="onescF")
            nc.vector.memset(onescF[:], 1.0)
            onesc = pers.tile([1, 128], SDT, tag="onesc")
            nc.vector.tensor_copy(onesc[:], onescF[:])

            ffcb_c = [pers.tile([DP, 1], F32, tag=f"ffcb{d}", name=f"ffcb{d}") for d in range(DT)]
            mq_t = [pers.tile([128, BL], F32, tag=f"mqt{qt}", name=f"mqt{qt}")
                    for qt in range(2)]

            # fc-phase persistents (loads deferred past layer-0 weights)
            tmpc = [fpers.tile([DP, BL], F32, tag=f"tmpc{d}", name=f"tmpc{d}")
                    for d in range(DT)]
            tmp1c = [fpers.tile([DP, BL], F32, tag=f"tmp1c{d}", name=f"tmp1c{d}")
                     for d in range(DT)]
            dfc = [fpers.tile([DP, BL], F32, tag=f"dfc{d}", name=f"dfc{d}")
                   for d in range(DT)]
            FAb = fpers.tile([FP, BL, FT], FCT, tag="FAb", name="FAb")
            FA = [FAb[:, :, i] for i in range(FT)]
            fTb = fpers.tile([FP, NCORES * BL, FT], FCT, tag="fTb",
                             name="fTb")
            ones32 = fpers.tile([1, B], F32, tag="ones32")
            nc.vector.memset(ones32[:], 1.0)
            ones = fpers.tile([1, B], FCT, tag="ones")
            nc.vector.tensor_copy(ones[:], ones32[:])
            with (
                tc.tile_pool(name="wattn", bufs=1) as wpool,
                tc.tile_pool(name="wrot", bufs=2) as wrot,
                tc.tile_pool(name="work", bufs=1) as wk_pool,
            ):
                def load_w(pool, src, l=None, tagp=None):
                    """Load a [D, D] pre-transposed weight as DT tiles."""
                    tiles = []
                    for d in range(DT):
                        tg = f"{tagp or src.name}{d}"
                        t = pool.tile([DP, D], SDT, tag=tg, name=tg)
                        ap = src.ap()[l] if l is not None else src.ap()
                        nc.sync.dma_start(t[:], ap[d * DP:(d + 1) * DP, :])
                        tiles.append(t)
                    return tiles

                def ps2k():
                    return psp.tile([128, 2 * S], F32, tag="ps256", bufs=2, name="ps2k")

                def ps1k():
                    return psp.tile([128, S], F32, tag="scps", bufs=4, name="ps1k")

                def softmax(ps, qt=None, j=None):
                    """scores psum -> normalized probs (SBUF). 4 chains in
                    flight: PE->DVE(max)->ACT(neg,exp+Z)->DVE(recip[,mask])
                    ->Pool(scale)."""
                    mx = wk_pool.tile([128, 1], F32, tag="mx", bufs=4, name="mx")
                    nc.vector.reduce_max(mx[:], ps[:], axis=AX)
                    ngm = wk_pool.tile([128, 1], F32, tag="ngm", bufs=4,
                                       name="ngm")
                    nc.scalar.mul(ngm[:], mx[:], -1.0)
                    probs = wk_pool.tile([128, S], F32, tag="probs", bufs=4,
                                         name="probs")
                    Z = wk_pool.tile([128, 1], F32, tag="Z", bufs=4, name="Z")
                    nc.scalar.activation(probs[:], ps[:], EXP, bias=ngm[:],
                                         scale=1.0, accum_out=Z[:])
                    r = wk_pool.tile([128, 1], F32, tag="r", bufs=4, name="r")
                    nc.vector.reciprocal(r[:], Z[:])
                    if qt is not None:
                        rm = wk_pool.tile([128, 1], F32, tag="rm", bufs=4,
                                          name="rm")
                        nc.vector.tensor_mul(rm[:], r[:], mq_t[qt][:, j:j + 1])
                        r = rm
                    nc.gpsimd.tensor_scalar_mul(probs[:], probs[:], r[:])
                    return probs

                drain_rr = [0]

                def drain(dst, src):
                    """PSUM->SBUF drain, rotated 2:1 ACT:DVE so psum rings
                    free even when one queue is backed up."""
                    i = drain_rr[0]
                    drain_rr[0] += 1
                    if i % 3 < 2:
                        nc.scalar.activation(dst, src, COPY)
                    else:
                        nc.vector.tensor_copy(dst, src)

                # ---------------- the 3-layer stack ----------------
                for l in range(K):
                    wq_sb = load_w(wpool, wq, l)   # M = scale * Wq^T Wk
                    wv_sb = load_w(wpool, wv, l)
                    if l == 0:
                        # deferred cold-start loads: first q/k matmuls only
                        # need Xt + wq/wk; everything here is used later
                        for d in range(DT):
                            nc.sync.dma_start(Xg[d][:],
                                              xt0[d * DP:(d + 1) * DP, :])
                        for d in range(DT):
                            nc.sync.dma_start(ffcb_c[d][:],
                                              ffcb[d * DP:(d + 1) * DP, :])
                        for qt in range(2):
                            nc.sync.dma_start(mq_t[qt][:], maskq.ap()[qt])
                        for d in range(DT):
                            nc.sync.dma_start(tmp1c[d][:],
                                              tmp1T[d * DP:(d + 1) * DP, :])
                    wo_sb = load_w(wrot, wo, l, tagp="wrot")
                    wffc_sb = load_w(wrot, wffc, tagp="wrot")

                    # ---- attention (per pair; 4 softmax chains batched) ----
                    for j in range(0, BL, 2):
                        pcols = slice(j * S, (j + 2) * S)
                        qT_t = []
                        for do in range(DT):
                            ps = ps2k()
                            for di in range(DT):
                                nc.tensor.matmul(
                                    ps[:DP, :],
                                    wq_sb[di][:, do * DP:(do + 1) * DP],
                                    Xt[di][:, pcols],
                                    start=(di == 0), stop=(di == DT - 1))
                            t = wk_pool.tile([DP, 2 * S], SDT, tag=f"qT{do}",
                                             name=f"qT{do}", bufs=QK_BUFS)
                            drain(t[:], ps[:DP, :])
                            qT_t.append(t)
                        # scores = (X M) X^T for both items x both q-halves
                        probs_l = {}
                        for jj in range(2):
                            off = jj * S
                            icols = slice((j + jj) * S, (j + jj + 1) * S)
                            for qt in range(2):
                                ps = ps1k()
                                for di in range(DT):
                                    qs = qT_t[di][:, off + qt * 128:
                                                  off + qt * 128 + 128]
                                    nc.tensor.matmul(
                                        ps[:], qs, Xt[di][:, icols],
                                        start=(di == 0), stop=(di == DT - 1))
                                probs_l[(jj, qt)] = softmax(ps)
                        # v for both items (PE filler during softmax chains)
                        v_pair = []
                        for jj in range(2):
                            v_sb = []
                            for st in range(2):
                                t = wk_pool.tile([128, D], SDT,
                                                 tag=f"v{2 * jj + st}",
                                                 name=f"v{2 * jj + st}")
                                scol = slice((j + jj) * S + st * 128,
                                             (j + jj) * S + st * 128 + 128)
                                for nt in range(2):
                                    ps = ps2k()
                                    for di in range(DT):
                                        nc.tensor.matmul(
                                            ps[:, :300], Xt[di][:, scol],
                                            wv_sb[di][:, nt * 300:(nt + 1) * 300],
                                            start=(di == 0), stop=(di == DT - 1))
                                    drain(t[:, nt * 300:(nt + 1) * 300],
                                          ps[:, :300])
                                v_sb.append(t)
                            v_pair.append(v_sb)
                        # prob transposes (paired drains) + attn + out proj
                        for jj in range(2):
                            cols = slice((j + jj) * S, (j + jj + 1) * S)
                            aTb = wk_pool.tile([128, 2 * S], SDT,
                                               tag=f"aTb{jj}", name=f"aTb{jj}")
                            aTv = aTb[:].rearrange("p (k q) -> p k q", k=2)
                            for qt in range(2):
                                probs = probs_l[(jj, qt)]
                                pt_ps = ps1k()
                                for kt in range(2):
                                    nc.tensor.transpose(
                                        pt_ps[:, kt * 128:(kt + 1) * 128],
                                        probs[:, kt * 128:(kt + 1) * 128],
                                        identF[:])
                                drain(aTv[:, :, qt * 128:(qt + 1) * 128],
                                      pt_ps[:].rearrange("p (k q) -> p k q",
                                                         k=2))
                            attnT = []
                            for d in range(DT):
                                ps = ps1k()
                                for kt in range(2):
                                    nc.tensor.matmul(
                                        ps[:DP, :],
                                        v_pair[jj][kt][:, d * DP:(d + 1) * DP],
                                        aTb[:, kt * S:(kt + 1) * S],
                                        start=(kt == 0), stop=(kt == 1))
                                t = wk_pool.tile([DP, S], SDT, tag=f"attnT{d}",
                                                 name=f"attnT{d}")
                                drain(t[:], ps[:DP, :])
                                attnT.append(t)
                            for do in range(DT):
                                ps = ps1k()
                                for di in range(DT):
                                    nc.tensor.matmul(
                                        ps[:DP, :],
                                        wo_sb[di][:, do * DP:(do + 1) * DP],
                                        attnT[di][:],
                                        start=(di == 0), stop=(di == DT - 1))
                                nc.vector.tensor_add(Xt[do][:, cols],
                                                     Xt[do][:, cols],
                                                     ps[:DP, :])

                    # ---- ffc (per pair; staged to dodge in-place hazard) ----
                    for j in range(0, BL, 2):
                        ccol = slice(j * S, (j + 2) * S)
                        stages = []
                        for do in range(DT):
                            ps = ps2k()
                            for di in range(DT):
                                nc.tensor.matmul(
                                    ps[:DP, :],
                                    wffc_sb[di][:, do * DP:(do + 1) * DP],
                                    Xt[di][:, ccol],
                                    start=(di == 0), stop=(di == DT - 1))
                            st = wk_pool.tile([128, 2 * S], SDT,
                                              tag=f"stg{do}",
                                              name=f"stg{do}")
                            if do % 3 < 2:
                                nc.scalar.activation(st[:DP, :], ps[:DP, :],
                                                     IDENT, bias=ffcb_c[do][:])
                            else:
                                nc.vector.tensor_scalar_add(st[:DP, :],
                                                            ps[:DP, :],
                                                            ffcb_c[do][:])
                            stages.append(st)
                        for do in range(DT):
                            nc.vector.tensor_add(Xt[do][:, ccol],
                                                 Xt[do][:, ccol],
                                                 stages[do][:DP, :])

                    # ---- graph conv (per item): Xg += relu((G/den) @ te) ----
                    wlin_sb = load_w(wrot, wlin, tagp="wrot")
                    for j in range(BL):
                        cols = slice(j * S, (j + 1) * S)
                        te_sb = []
                        for st in range(2):
                            t = wk_pool.tile([128, D], SDT, tag=f"v{st}",
                                             name=f"te{st}")
                            scol = slice(j * S + st * 128,
                                         j * S + st * 128 + 128)
                            for nt in range(2):
                                ps = ps2k()
                                for di in range(DT):
                                    nc.tensor.matmul(
                                        ps[:, :300], Xg[di][:, scol],
                                        wlin_sb[di][:, nt * 300:(nt + 1) * 300],
                                        start=(di == 0), stop=(di == DT - 1))
                                drain(t[:, nt * 300:(nt + 1) * 300],
                                      ps[:, :300])
                            te_sb.append(t)
                        g_sb = []
                        for kt in range(2):
                            t = wk_pool.tile([128, S], SDT, tag=f"sh{kt}",
                                             name=f"g{kt}")
                            nc.sync.dma_start(
                                t[:], gts.ap()[j, kt * 128:(kt + 1) * 128, :])
                            g_sb.append(t)
                        for d in range(DT):
                            ps = ps1k()
                            for kt in range(2):
                                nc.tensor.matmul(
                                    ps[:DP, :],
                                    te_sb[kt][:, d * DP:(d + 1) * DP],
                                    g_sb[kt][:], start=(kt == 0),
                                    stop=(kt == 1))
                            rl = wk_pool.tile([DP, S], F32, tag="rl",
                                              name="rl", bufs=2)
                            nc.scalar.activation(rl[:], ps[:DP, :], RELU)
                            nc.vector.tensor_add(Xg[d][:, cols],
                                                 Xg[d][:, cols], rl[:])

                    # ---- mutual biaffine (per item; chains batched) ----
                    wb_sb = load_w(wrot, wbiaff, tagp="wrot")
                    pqkT = {}
                    for j in range(BL):
                        cols = slice(j * S, (j + 1) * S)
                        if j % 2 == 0:
                            pcols = slice(j * S, (j + 2) * S)
                            pqkT = {"q": [], "k": []}
                            for (xsrc, nm) in ((Xt, "q"), (Xg, "k")):
                                for do in range(DT):
                                    ps = ps2k()
                                    for di in range(DT):
                                        nc.tensor.matmul(
                                            ps[:DP, :],
                                            wb_sb[di][:, do * DP:(do + 1) * DP],
                                            xsrc[di][:, pcols],
                                            start=(di == 0),
                                            stop=(di == DT - 1))
                                    t = wk_pool.tile([DP, 2 * S], SDT,
                                                     tag=f"{nm}T{do}",
                                                     name=f"{nm}T{do}",
                                                     bufs=QK_BUFS)
                                    drain(t[:], ps[:DP, :])
                                    pqkT[nm].append(t)
                        off = (j % 2) * S
                        negrow = wk_pool.tile([1, S], SDT, tag="negrow",
                                              name="negrow", bufs=2)
                        nc.sync.dma_start(negrow[:], negmask.ap()[j])
                        # l1/l2 scores (+rank-1 neg mask preload), 4 chains
                        probs_l = {}
                        for (pnm, xrhs, nm) in (("q", Xg, "l1"),
                                                ("k", Xt, "l2")):
                            for qt in range(2):
                                ps = ps1k()
                                nc.tensor.matmul(ps[:], onesc[:, :128],
                                                 negrow[:], start=True,
                                                 stop=False)
                                for di in range(DT):
                                    pv = pqkT[pnm][di][:, off + qt * 128:
                                                       off + qt * 128 + 128]
                                    nc.tensor.matmul(
                                        ps[:], pv, xrhs[di][:, cols],
                                        start=False, stop=(di == DT - 1))
                                probs_l[(nm, qt)] = softmax(ps, qt, j)
                        # natural-layout Xt/Xg (PE filler during chains)
                        natXt, natXg = [], []
                        for (X, nat, base) in ((Xt, natXt, 0), (Xg, natXg, 2)):
                            for st in range(2):
                                t = wk_pool.tile([128, D], SDT,
                                                 tag=f"v{base + st}",
                                                 name=f"nat{base + st}")
                                scol = slice(j * S + st * 128,
                                             j * S + st * 128 + 128)
                                for d0 in range(0, DT, 2):
                                    dn = min(2, DT - d0)
                                    pt_ps = psp.tile([128, S], SDT,
                                                     tag="psT", bufs=2,
                                                     name="psTn")
                                    for dd in range(dn):
                                        nc.tensor.transpose(
                                            pt_ps[:, dd * DP:(dd + 1) * DP],
                                            X[d0 + dd][:, scol],
                                            identR[:DP, :DP])
                                    drain(t[:, d0 * DP:(d0 + dn) * DP],
                                          pt_ps[:, :dn * DP])
                                nat.append(t)
                        # prob transposes (paired drains)
                        lTb = {}
                        for nm in ("l1", "l2"):
                            tb = wk_pool.tile([128, 2 * S], SDT,
                                              tag=f"aTb{(nm == 'l2') * 1}",
                                              name=f"lTb{nm}")
                            tv = tb[:].rearrange("p (k q) -> p k q", k=2)
                            for qt in range(2):
                                probs = probs_l[(nm, qt)]
                                pt_ps = ps1k()
                                for kt in range(2):
                                    nc.tensor.transpose(
                                        pt_ps[:, kt * 128:(kt + 1) * 128],
                                        probs[:, kt * 128:(kt + 1) * 128],
                                        identF[:])
                                drain(tv[:, :, qt * 128:(qt + 1) * 128],
                                      pt_ps[:].rearrange("p (k q) -> p k q",
                                                         k=2))
                            lTb[nm] = tb
                        # o1 into Xt, o2 into Xg (q-mask folded into rm)
                        for (nat, lname, X) in ((natXg, "l1", Xt),
                                                (natXt, "l2", Xg)):
                            for d in range(DT):
                                ps = ps1k()
                                for kt in range(2):
                                    nc.tensor.matmul(
                                        ps[:DP, :],
                                        nat[kt][:, d * DP:(d + 1) * DP],
                                        lTb[lname][:, kt * S:(kt + 1) * S],
                                        start=(kt == 0), stop=(kt == 1))
                                nc.vector.tensor_add(X[d][:, cols],
                                                     X[d][:, cols],
                                                     ps[:DP, :])
                        # span sum of out_t for this item (last layer only)
                        if l == K - 1:
                            ws_bc = wk_pool.tile([128, S], SDT, tag="nmbc",
                                                 name="ws_bc")
                            nc.sync.dma_start(
                                ws_bc[:],
                                wspan.ap()[j].partition_broadcast(128))
                            for d in range(DT):
                                msel = wk_pool.tile([DP, S], F32, tag="msel",
                                                    name="msel")
                                nc.gpsimd.tensor_mul(msel[:], Xt[d][:, cols],
                                                     ws_bc[:DP, :])
                                nc.vector.reduce_sum(tmpc[d][:, j:j + 1],
                                                     msel[:], axis=AX)

                # feature blocks: [hout, tmp, tmp1, tmp*tmp1, |tmp-tmp1|]
                for d in range(DT):
                    nc.gpsimd.dma_start(FA[d], houtT[d * DP:(d + 1) * DP, :])
                    nc.gpsimd.tensor_copy(FA[5 + d], tmpc[d][:])
                    nc.gpsimd.tensor_copy(FA[10 + d], tmp1c[d][:])
                    nc.gpsimd.tensor_mul(FA[15 + d], tmpc[d][:],
                                         tmp1c[d][:])
                    nc.gpsimd.tensor_sub(dfc[d][:], tmpc[d][:], tmp1c[d][:])
                    nc.scalar.activation(FA[20 + d], dfc[d][:], ABS)

            # ---------------- FC: out = feat @ fc_w.T + fc_b ----------------
            with tc.tile_pool(name="fc", bufs=2) as fcp:
                fcw_v = fcw.ap().rearrange("(f p) o -> p f o", p=FP)

                def load_wg(oc):
                    w = _och(oc)
                    wg = fcp.tile([FP, FT, OCH], FCT, tag="wg", name="wg")
                    for f0, f1 in ((0, 7), (7, 13), (13, 19), (19, 25)):
                        nc.sync.dma_start(
                            wg[:, f0:f1, :w],
                            fcw_v[:, f0:f1, oc * OCH:oc * OCH + w])
                    fcbc = fcp.tile([1, OCH], FCT, tag="fcbc", name="fcbc")
                    nc.sync.dma_start(fcbc[:, :w],
                                      fcb[:, oc * OCH:oc * OCH + w])
                    return wg, fcbc

                # prefetch the first two weight chunks; they overlap the
                # feature AllGather below
                wg_pre = [load_wg(0), load_wg(1)]

                # ---------------- allgather features ----------------
                feat_l = dpool.tile([FP, BL, FT], FCT)
                nc.sync.dma_start(feat_l[:], FAb[:])
                feat_g = dpool.tile([NCORES, FP, BL, FT], FCT)
                if NO_CC:
                    nc.sync.dma_start(feat_g[0], feat_l[:])
                else:
                    nc.gpsimd.collective_compute(
                        "AllGather", mybir.AluOpType.bypass,
                        replica_groups=[list(range(NCORES))],
                        ins=[feat_l.opt()], outs=[feat_g.opt()])
                # [c, p, j, f] -> [p, c, j, f]: contiguous (j f) runs of 400B
                gview = feat_g[:].rearrange("c p j f -> p c j f")
                fTv = fTb[:].rearrange("p (c j) f -> p c j f", c=NCORES)
                nc.sync.dma_start(fTv, gview)

                for oc in range(NOC):
                    w = _och(oc)
                    wg, fcbc = wg_pre[oc] if oc < 2 else load_wg(oc)
                    ps = psp.tile([128, OCH], F32, tag="ps256", bufs=2, name="psfc")
                    for i in range(FT):
                        nc.tensor.matmul(
                            ps[:B, :w], fTb[:, :, i],
                            wg[:, i, :w], start=(i == 0), stop=False)
                    nc.tensor.matmul(ps[:B, :w], ones[:], fcbc[:, :w],
                                     start=False, stop=True)
                    ot = fcp.tile([B, OCH], F32, tag="ot", name="ot")
                    nc.vector.tensor_copy(ot[:, :w], ps[:B, :w])
                    nc.sync.dma_start(out[:, oc * OCH:oc * OCH + w],
                                      ot[:, :w])

    nc.compile()
    return nc


def prep_inputs(lstm_out, hout, dependency_graph, attn_in, attn_out, ffc_w,
                ffc_b, lin_w, biaff_w, fc_w, fc_b, text_len, spans):
    """Host-side sharding + layout transforms. Returns per-core input maps."""
    f32 = np.float32
    lstm_out = np.asarray(lstm_out, dtype=f32)
    hout = np.asarray(hout, dtype=f32)
    G = np.asarray(dependency_graph, dtype=f32)
    attn_in = np.asarray(attn_in, dtype=f32)
    attn_out = np.asarray(attn_out, dtype=f32)
    fc_w = np.asarray(fc_w, dtype=f32)
    text_len = np.asarray(text_len)
    spans = np.asarray(spans)

    scale = 1.0 / math.sqrt(D)
    # fold q/k into one projection: scores = (X Wq^T)(Wk X^T) = X M X^T
    wq = np.ascontiguousarray(np.stack([
        (attn_in[l, :D, :].T.astype(np.float64) * scale)
        @ attn_in[l, D:2 * D, :].astype(np.float64)
        for l in range(K)]).astype(f32))
    wv = np.ascontiguousarray(np.stack([attn_in[l, 2 * D:, :].T
                                        for l in range(K)]))
    wo = np.ascontiguousarray(np.stack([attn_out[l].T for l in range(K)]))
    wffc = np.ascontiguousarray(np.asarray(ffc_w, dtype=f32).T)
    wlin = np.ascontiguousarray(np.asarray(lin_w, dtype=f32).T)
    wbiaff = np.ascontiguousarray(np.asarray(biaff_w, dtype=f32).T)
    ffcb = np.ascontiguousarray(np.asarray(ffc_b, dtype=f32).reshape(D, 1))
    fcb = np.asarray(fc_b, dtype=f32).reshape(1, OUT1)

    idx = np.arange(S)
    mask = (idx[None, :] < text_len[:, None].astype(np.int64)).astype(f32)
    negm = (-10000.0 * (1.0 - mask)).reshape(B, 1, S)
    maskq_h = mask.reshape(B, 2, 128)
    s0 = spans[:, 0, 0].astype(np.int64)[:, None]
    e0 = spans[:, 0, 1].astype(np.int64)[:, None]
    wsp = ((idx[None, :] >= s0) & (idx[None, :] < e0)).astype(f32)
    tmp1 = np.einsum('bs,bsd->bd', wsp, lstm_out)  # span_sum(lstm_out)[:, 0]
    wsp = wsp.reshape(B, 1, S)

    denom = G.sum(axis=2, keepdims=True) + 1e-7
    GTs = np.ascontiguousarray((G / denom).transpose(0, 2, 1))

    in_maps = []
    for c in range(NCORES):
        bs = slice(c * BL, (c + 1) * BL)
        xt0 = np.ascontiguousarray(
            lstm_out[bs].transpose(2, 0, 1).reshape(D, NS))
        in_maps.append({
            "xt0": xt0,
            "gts": np.ascontiguousarray(GTs[bs]),
            "negmask": np.ascontiguousarray(negm[bs]),
            "maskq": np.ascontiguousarray(maskq_h[bs].transpose(1, 2, 0)),
            "wspan": np.ascontiguousarray(wsp[bs]),
            "houtT": np.ascontiguousarray(hout[bs].T).astype(ml_dtypes.bfloat16),
            "tmp1T": np.ascontiguousarray(tmp1[bs].T),
            "wq": wq, "wv": wv, "wo": wo,
            "wffc": wffc, "wlin": wlin, "wbiaff": wbiaff, "ffcb": ffcb,
            "fcw": np.ascontiguousarray(
                fc_w[c * OSH:(c + 1) * OSH, :].T).astype(ml_dtypes.bfloat16),
            "fcb": np.ascontiguousarray(
                fcb[:, c * OSH:(c + 1) * OSH]).astype(ml_dtypes.bfloat16),
        })
    return in_maps


_NC = None


def get_nc():
    global _NC
    if _NC is None:
        _NC = build_nc()
    return _NC


def kernel(**inputs) -> np.ndarray:
    nc = get_nc()
    in_maps = prep_inputs(**inputs)
    res = run_bass_kernel_spmd(nc, in_maps, list(range(NCORES)))
    return np.concatenate([res.results[c]["out"] for c in range(NCORES)],
                          axis=1)
